# revision 30
# baseline (speedup 1.0000x reference)
"""Causal self-attention (B=4, T=1024, D=1024, H=16) on 8 Trainium2 NeuronCores.

Zero-collective sharding: heads 2c,2c+1 -> core c (head/tensor parallel).
Each core computes q,k projections for its 2 heads in transposed layout
[f, t] (from x^T), v directly in [t, f] layout via a flipped GEMM whose
weight block carries two zero columns (so one fused DVE add installs the
v bias and the ones columns that harvest softmax row-sums), causal
softmax attention with the key-mask folded into an extra contraction
row, then a PARTIAL output projection using only its 128 rows of Wproj:
out_partial = att_local^T @ Wproj[c*128:(c+1)*128, :] for all 4096
tokens. The host sums the 8 partials (+ bproj) during unshard — no
device collectives, so per-core spans are independent of launch skew.
bf16 matmul operands, fp32 PSUM accumulate. Next-batch x/q/k tiles are
prefetched ahead of the attention phase so their DMAs are not stuck
behind the output stores on the sync queue.
"""
import numpy as np

B, T, D, H = 4, 1024, 1024, 16
DH = D // H  # 64
NC = 8
HPC = H // NC  # 2 heads per core
WCH = 2 * 128 + 130  # per-d-chunk weight block: wq(128) wk(128) wv_aug(130)

_CACHE = {}


def _chunks(width):
    out, off, rem = [], 0, width
    while rem > 0:
        w = min(rem, 512)
        out.append((off, w))
        off += w
        rem -= w
    return out


def _build():
    import concourse.mybir as mybir
    import concourse.tile as tile
    from concourse import bacc

    F32 = mybir.dt.float32
    BF16 = mybir.dt.bfloat16
    U16 = mybir.dt.uint16
    EXP = mybir.ActivationFunctionType.Exp
    RECIP = mybir.ActivationFunctionType.Reciprocal
    COPY = mybir.ActivationFunctionType.Copy
    MULT = mybir.AluOpType.mult
    ADD = mybir.AluOpType.add

    nc = bacc.Bacc("TRN2", target_bir_lowering=False, debug=False, num_devices=NC)

    xt_d = nc.dram_tensor("xt", [B, 8, 128, T], BF16, kind="ExternalInput").ap()
    wqkv_d = nc.dram_tensor("wqkv", [128, 8 * WCH], BF16, kind="ExternalInput").ap()
    bqb_d = nc.dram_tensor("bqb", [128, T], BF16, kind="ExternalInput").ap()
    bkb_d = nc.dram_tensor("bkb", [128, T], BF16, kind="ExternalInput").ap()
    bvb_d = nc.dram_tensor("bvb", [128, 130], F32, kind="ExternalInput").ap()
    biask_d = nc.dram_tensor("biask", [B, T], BF16, kind="ExternalInput").ap()
    wproj_d = nc.dram_tensor("wproj", [128, D], BF16, kind="ExternalInput").ap()
    tri_d = nc.dram_tensor("tri", [128, 128], BF16, kind="ExternalInput").ap()
    out_d = nc.dram_tensor("out", [B * T, D], BF16, kind="ExternalOutput").ap()

    with tile.TileContext(nc) as tc:
        with (
            tc.tile_pool(name="consts", bufs=1) as cpool,
            tc.tile_pool(name="xt", bufs=8) as xt_pool,
            tc.tile_pool(name="xtb", bufs=2) as xtb_pool,
            tc.tile_pool(name="qk", bufs=2) as qk_pool,
            tc.tile_pool(name="vs", bufs=16) as vs_pool,
            tc.tile_pool(name="att", bufs=10) as att_pool,
            tc.tile_pool(name="fin", bufs=2) as fin_pool,
            tc.tile_pool(name="nrm", bufs=4) as nrm_pool,
            tc.tile_pool(name="ysb", bufs=4) as y_pool,
            tc.tile_pool(name="mmps", bufs=2, space="PSUM") as mm_ps_pool,
            tc.tile_pool(name="yps", bufs=2, space="PSUM") as y_ps_pool,
            tc.tile_pool(name="sps", bufs=2, space="PSUM") as s_ps_pool,
            tc.tile_pool(name="ops", bufs=2, space="PSUM") as o_ps_pool,
        ):
            # ---- tiles for constants / weights ----
            tri = cpool.tile([128, 128], BF16, name="tri", tag="tri")
            bvb = cpool.tile([128, 130], F32, name="bvb", tag="bvb")
            wq_sb = cpool.tile([128, 8 * WCH], BF16, name="wq", tag="wq")
            wp_sb = cpool.tile([128, D], BF16, name="wp", tag="wp")
            bqb = cpool.tile([128, T], BF16, name="bqb", tag="bqb")
            bkb = cpool.tile([128, T], BF16, name="bkb", tag="bkb")

            def load_qk(b):
                """Allocate + start filling next batch's q/k bias rows."""
                qt = [
                    qk_pool.tile([DH + 1, T], BF16, name=f"qt{h}", tag=f"qt{h}")
                    for h in range(HPC)
                ]
                kt = [
                    qk_pool.tile([DH + 1, T], BF16, name=f"kt{h}", tag=f"kt{h}")
                    for h in range(HPC)
                ]
                for h in range(HPC):
                    if b < 2:  # qk pool has 2 buffers; row 64 persists after
                        nc.vector.memset(qt[h][DH : DH + 1, :].bitcast(U16), 0x3F80)
                    nc.sync.dma_start(kt[h][DH : DH + 1, :], biask_d[b : b + 1, :])
                return qt, kt

            def load_xt(b):
                # one coalesced DMA: chunk i of x^T lands at cols [i*T,(i+1)*T)
                xt_t = xtb_pool.tile([128, 8 * T], BF16, name="xtb", tag="xtb")
                nc.sync.dma_start(
                    xt_t[:].rearrange("p (i t) -> p i t", i=8),
                    xt_d[b].rearrange("i p t -> p i t"),
                )
                return xt_t

            # weights/x first (the first qk chain needs wq half 0 + x chunk
            # 0); consts later — nothing reads them for ~15us
            first_xt = []
            nc.sync.dma_start(wq_sb[:, 0 : 4 * WCH], wqkv_d[:, 0 : 4 * WCH])
            for i in range(4):
                xt_t = xt_pool.tile([128, T], BF16, name="xt", tag="xt")
                nc.sync.dma_start(xt_t[:], xt_d[0, i, :, :])
                first_xt.append(xt_t)
            nc.sync.dma_start(wq_sb[:, 4 * WCH :], wqkv_d[:, 4 * WCH :])
            for i in range(4, 8):
                xt_t = xt_pool.tile([128, T], BF16, name="xt", tag="xt")
                nc.sync.dma_start(xt_t[:], xt_d[0, i, :, :])
                first_xt.append(xt_t)
            nc.sync.dma_start(bqb[:], bqb_d[:])
            nc.sync.dma_start(bkb[:], bkb_d[:])
            nc.sync.dma_start(bvb[:], bvb_d[:])
            nc.sync.dma_start(tri[:], tri_d[:])
            nc.sync.dma_start(wp_sb[:], wproj_d[:])

            # warm the Act exp table while startup DMAs are in flight
            warm = cpool.tile([1, 1], F32, name="warm", tag="warm")
            nc.scalar.activation(warm[:], warm[:], EXP)

            cur_xt = first_xt
            cur_qt, cur_kt = load_qk(0)

            def proj(fin, pb):
                """Partial output projection for batch pb's 1024 rows.
                Drains alternate Act/DVE; one coalesced store per tb."""
                for tb in range(8):
                    y_sb = y_pool.tile([128, T], BF16, name="ysb", tag="ysb")
                    for ch in range(2):
                        csl = slice(ch * 512, (ch + 1) * 512)
                        y_ps = y_ps_pool.tile([128, 512], F32, name="yps", tag="yps")
                        nc.tensor.matmul(
                            y_ps[:],
                            fin[:, tb * 128 : (tb + 1) * 128],
                            wp_sb[:, csl],
                            start=True,
                            stop=True,
                        )
                        if ch == 0:
                            nc.vector.tensor_copy(y_sb[:, csl], y_ps[:])
                        else:
                            nc.scalar.activation(y_sb[:, csl], y_ps[:], COPY)
                    nc.sync.dma_start(
                        out_d[pb * T + tb * 128 : pb * T + (tb + 1) * 128, :],
                        y_sb[:],
                    )

            def norm_ch(h, o_ps, att_fin, ch):
                """Normalize o_ps[ch] rows by the folded row-sum (row DH).
                NB: DVE ops straight off a PSUM row at partition offset 64
                read the wrong row — bounce through Act."""
                csl = slice(ch * 512, (ch + 1) * 512)
                ssum = nrm_pool.tile([1, 512], F32, name="ssum", tag="ssum")
                recip = nrm_pool.tile([1, 512], F32, name="recip", tag="recip")
                bcast = nrm_pool.tile([DH, 512], F32, name="bcast", tag="bcast")
                nc.scalar.activation(ssum[:], o_ps[ch][DH : DH + 1, :], COPY)
                nc.vector.reciprocal_approx_fast(recip[:], ssum[:])
                nc.gpsimd.partition_broadcast(bcast[:], recip[:])
                nc.vector.tensor_tensor(
                    out=att_fin[h * DH : (h + 1) * DH, csl],
                    in0=o_ps[ch][0:DH, :],
                    in1=bcast[:],
                    op=MULT,
                )

            prev_fin, prev_b = None, None

            for b in range(B):
                xt_sb = cur_xt
                qt, kt = cur_qt, cur_kt
                if b == 0:
                    xs = lambda i, lo, hi: xt_sb[i][:, lo:hi]
                else:
                    xs = lambda i, lo, hi: xt_sb[:, i * T + lo : i * T + hi]

                # ---- q,k projections (transposed: [f, t]) ----
                for fb in range(2):
                    bias = bqb if fb == 0 else bkb
                    dst = qt if fb == 0 else kt
                    for ch in range(2):
                        csl = slice(ch * 512, (ch + 1) * 512)
                        mm_ps = mm_ps_pool.tile(
                            [128, 512], F32, name="mm512", tag="mm512"
                        )
                        for i in range(8):
                            nc.tensor.matmul(
                                mm_ps[:],
                                wq_sb[:, i * WCH + fb * 128 : i * WCH + (fb + 1) * 128],
                                xs(i, ch * 512, (ch + 1) * 512),
                                start=(i == 0),
                                stop=(i == 7),
                            )
                        for h in range(HPC):
                            nc.vector.tensor_tensor(
                                out=dst[h][0:DH, csl],
                                in0=mm_ps[h * DH : (h + 1) * DH, :],
                                in1=bias[h * DH : (h + 1) * DH, csl],
                                op=ADD,
                            )

                # ---- prefetch next batch's x / qk tiles first: their DMA
                # triggers must not sit behind the proj stores' semaphore
                # waits on the sync queue ----
                if b + 1 < B:
                    cur_xt = load_xt(b + 1)
                    cur_qt, cur_kt = load_qk(b + 1)

                # ---- proj of the previous batch: ready PE work that hides
                # the previous normalize + drain latency ----
                if prev_fin is not None:
                    proj(prev_fin, prev_b)

                # ---- v directly in [t, f] layout ----
                v_sb = []
                for tb in range(8):
                    mm_ps = mm_ps_pool.tile([128, 512], F32, name="mm512", tag="mm512")
                    for i in range(8):
                        nc.tensor.matmul(
                            mm_ps[:, 0:130],
                            xs(i, tb * 128, (tb + 1) * 128),
                            wq_sb[:, i * WCH + 256 : (i + 1) * WCH],
                            start=(i == 0),
                            stop=(i == 7),
                        )
                    vs = vs_pool.tile([128, 130], BF16, name="vsb", tag="vsb")
                    nc.vector.tensor_tensor(
                        out=vs[:], in0=mm_ps[:, 0:130], in1=bvb[:], op=ADD
                    )
                    v_sb.append(vs)

                # ---- attention per head ----
                att_fin = fin_pool.tile([128, T], BF16, name="fin", tag="fin")
                for h in range(HPC):
                    o_ps = [
                        o_ps_pool.tile([DH + 1, 512], F32, name="o_ps", tag="o_ps")
                        for _ in range(2)
                    ]
                    atts = []

                    def av(kb):
                        # AV lags scores by 2 kb-steps: exp+tri latency is
                        # hidden without draining the PE queue
                        k0 = kb * 128
                        att = atts[kb]
                        for ch in range(2):
                            lo = max(k0, ch * 512)
                            hi = (ch + 1) * 512
                            if lo >= hi:
                                continue
                            nc.tensor.matmul(
                                o_ps[ch][:, lo - ch * 512 : hi - ch * 512],
                                v_sb[kb][:, h * 65 : (h + 1) * 65],
                                att[:, lo - k0 : hi - k0],
                                start=(kb == 0),
                                stop=(kb == (3 if ch == 0 else 7)),
                            )

                    for kb in range(8):
                        k0 = kb * 128
                        width = T - k0
                        att = att_pool.tile([128, T], BF16, name="att", tag="att")
                        for off, w in _chunks(width):
                            s_ps = s_ps_pool.tile(
                                [128, 512], F32, name="s_ps", tag="s_ps"
                            )
                            nc.tensor.matmul(
                                s_ps[:, 0:w],
                                kt[h][:, k0 : k0 + 128],
                                qt[h][:, k0 + off : k0 + off + w],
                                start=True,
                                stop=True,
                            )
                            nc.scalar.activation(
                                att[:, off : off + w], s_ps[:, 0:w], EXP
                            )
                        # causal mask on the diagonal block
                        nc.vector.tensor_tensor(
                            out=att[:, 0:128],
                            in0=att[:, 0:128],
                            in1=tri[:],
                            op=MULT,
                        )
                        atts.append(att)
                        if kb >= 4:
                            av(kb - 4)
                    av(4)
                    av(5)
                    av(6)
                    av(7)
                    norm_ch(h, o_ps, att_fin, 0)
                    norm_ch(h, o_ps, att_fin, 1)

                prev_fin, prev_b = att_fin, b

            proj(prev_fin, prev_b)

    nc.compile()
    return nc


def _get_nc():
    if "nc" not in _CACHE:
        _CACHE["nc"] = _build()
    return _CACHE["nc"]


def kernel(x, Wqkv, bqkv, Wproj, bproj, mask):
    import ml_dtypes
    from concourse.bass_utils import run_bass_kernel_spmd

    BF = ml_dtypes.bfloat16
    x = np.asarray(x, dtype=np.float32)
    Wqkv = np.asarray(Wqkv, dtype=np.float32)
    bqkv = np.asarray(bqkv, dtype=np.float32)
    Wproj = np.asarray(Wproj, dtype=np.float32)
    bproj = np.asarray(bproj, dtype=np.float32)
    mask = np.asarray(mask)

    nc = _get_nc()

    # [B, 8, 128, T]: d-chunk-major view of x^T
    xt = np.ascontiguousarray(x.transpose(0, 2, 1)).astype(BF).reshape(B, 8, 128, T)
    biask = np.where(mask == 0, np.float32(-30000.0), np.float32(0.0)).astype(BF)
    tri = np.triu(np.ones((128, 128), np.float32)).astype(BF)

    in_maps = []
    for c in range(NC):
        cols = slice(c * HPC * DH, (c + 1) * HPC * DH)  # this core's head features
        wq = Wqkv[:, 0:D][:, cols] * 0.125  # scores scale folded into Wq/bq
        wk = Wqkv[:, D : 2 * D][:, cols]
        wv = Wqkv[:, 2 * D : 3 * D][:, cols]
        # per-chunk layout [wq(128) | wk(128) | wv_h0(64) 0 wv_h1(64) 0]
        wv_aug = np.zeros((D, 130), np.float32)
        wv_aug[:, 0:DH] = wv[:, 0:DH]
        wv_aug[:, DH + 1 : 2 * DH + 1] = wv[:, DH : 2 * DH]
        # pre-chunked [128, 8*WCH]: chunk i rows i*128:(i+1)*128 side by side
        w_local = (
            np.concatenate([wq, wk, wv_aug], axis=1)
            .astype(BF)
            .reshape(8, 128, WCH)
            .transpose(1, 0, 2)
            .reshape(128, 8 * WCH)
        )
        w_local = np.ascontiguousarray(w_local)
        bq = bqkv[0:D][cols] * 0.125
        bk = bqkv[D : 2 * D][cols]
        bv = bqkv[2 * D : 3 * D][cols]
        bqb = np.repeat(bq[:, None], T, axis=1).astype(BF)  # [128, T]
        bkb = np.repeat(bk[:, None], T, axis=1).astype(BF)
        bvrow = np.zeros(130, np.float32)
        bvrow[0:DH] = bv[0:DH]
        bvrow[DH] = 1.0
        bvrow[DH + 1 : 2 * DH + 1] = bv[DH : 2 * DH]
        bvrow[2 * DH + 1] = 1.0
        bvb = np.broadcast_to(bvrow, (128, 130)).copy()
        in_maps.append(
            {
                "xt": xt,
                "wqkv": w_local,
                "bqb": bqb,
                "bkb": bkb,
                "bvb": bvb,
                "biask": biask,
                "wproj": Wproj[c * 128 : (c + 1) * 128, :].astype(BF),
                "tri": tri,
            }
        )

    res = run_bass_kernel_spmd(nc, in_maps, core_ids=list(range(NC)))
    # out_c = att_local^T @ Wproj_local; full out = sum_c out_c + bproj
    y = res.results[0]["out"].astype(np.float32)
    for c in range(1, NC):
        y += res.results[c]["out"].astype(np.float32)
    y = (y + bproj).astype(np.float32)
    return y.reshape(B, T, D)


# revision 31
# speedup vs baseline: 1.0207x; 1.0207x over previous
"""Causal self-attention (B=4, T=1024, D=1024, H=16) on 8 Trainium2 NeuronCores.

Zero-collective sharding: heads 2c,2c+1 -> core c (head/tensor parallel).
Each core computes q,k projections for its 2 heads in transposed layout
[f, t] (from x^T), v directly in [t, f] layout via a flipped GEMM whose
weight block carries two zero columns (so one fused DVE add installs the
v bias and the ones columns that harvest softmax row-sums), causal
softmax attention with the key-mask folded into an extra contraction
row, then a PARTIAL output projection using only its 128 rows of Wproj:
out_partial = att_local^T @ Wproj[c*128:(c+1)*128, :] for all 4096
tokens. The host sums the 8 partials (+ bproj) during unshard — no
device collectives, so per-core spans are independent of launch skew.
bf16 matmul operands, fp32 PSUM accumulate. Next-batch x/q/k tiles are
prefetched ahead of the attention phase so their DMAs are not stuck
behind the output stores on the sync queue.
"""
import numpy as np

B, T, D, H = 4, 1024, 1024, 16
DH = D // H  # 64
NC = 8
HPC = H // NC  # 2 heads per core
WCH = 2 * 128 + 130  # per-d-chunk weight block: wq(128) wk(128) wv_aug(130)

_CACHE = {}


def _chunks(width):
    out, off, rem = [], 0, width
    while rem > 0:
        w = min(rem, 512)
        out.append((off, w))
        off += w
        rem -= w
    return out


def _build():
    import concourse.mybir as mybir
    import concourse.tile as tile
    from concourse import bacc

    F32 = mybir.dt.float32
    BF16 = mybir.dt.bfloat16
    U16 = mybir.dt.uint16
    EXP = mybir.ActivationFunctionType.Exp
    RECIP = mybir.ActivationFunctionType.Reciprocal
    COPY = mybir.ActivationFunctionType.Copy
    MULT = mybir.AluOpType.mult
    ADD = mybir.AluOpType.add

    nc = bacc.Bacc("TRN2", target_bir_lowering=False, debug=False, num_devices=NC)

    xt_d = nc.dram_tensor("xt", [B, 8, 128, T], BF16, kind="ExternalInput").ap()
    wqkv_d = nc.dram_tensor("wqkv", [128, 8 * WCH], BF16, kind="ExternalInput").ap()
    bqb_d = nc.dram_tensor("bqb", [128, T], BF16, kind="ExternalInput").ap()
    bkb_d = nc.dram_tensor("bkb", [128, T], BF16, kind="ExternalInput").ap()
    bvb_d = nc.dram_tensor("bvb", [128, 130], F32, kind="ExternalInput").ap()
    biask_d = nc.dram_tensor("biask", [B, T], BF16, kind="ExternalInput").ap()
    wproj_d = nc.dram_tensor("wproj", [128, D], BF16, kind="ExternalInput").ap()
    tri_d = nc.dram_tensor("tri", [128, 128], BF16, kind="ExternalInput").ap()
    out_d = nc.dram_tensor("out", [B * T, D], BF16, kind="ExternalOutput").ap()

    with tile.TileContext(nc) as tc:
        with (
            tc.tile_pool(name="consts", bufs=1) as cpool,
            tc.tile_pool(name="xt", bufs=8) as xt_pool,
            tc.tile_pool(name="xtb", bufs=2) as xtb_pool,
            tc.tile_pool(name="qk", bufs=2) as qk_pool,
            tc.tile_pool(name="vs", bufs=16) as vs_pool,
            tc.tile_pool(name="att", bufs=10) as att_pool,
            tc.tile_pool(name="fin", bufs=2) as fin_pool,
            tc.tile_pool(name="nrm", bufs=4) as nrm_pool,
            tc.tile_pool(name="ysb", bufs=4) as y_pool,
            tc.tile_pool(name="mmps", bufs=2, space="PSUM") as mm_ps_pool,
            tc.tile_pool(name="yps", bufs=2, space="PSUM") as y_ps_pool,
            tc.tile_pool(name="sps", bufs=2, space="PSUM") as s_ps_pool,
            tc.tile_pool(name="ops", bufs=2, space="PSUM") as o_ps_pool,
        ):
            # ---- tiles for constants / weights ----
            tri = cpool.tile([128, 128], BF16, name="tri", tag="tri")
            bvb = cpool.tile([128, 130], F32, name="bvb", tag="bvb")
            wq_sb = cpool.tile([128, 8 * WCH], BF16, name="wq", tag="wq")
            wp_sb = cpool.tile([128, D], BF16, name="wp", tag="wp")
            bqb = cpool.tile([128, T], BF16, name="bqb", tag="bqb")
            bkb = cpool.tile([128, T], BF16, name="bkb", tag="bkb")

            def load_qk(b):
                """Allocate + start filling next batch's q/k bias rows."""
                qt = [
                    qk_pool.tile([DH + 1, T], BF16, name=f"qt{h}", tag=f"qt{h}")
                    for h in range(HPC)
                ]
                kt = [
                    qk_pool.tile([DH + 1, T], BF16, name=f"kt{h}", tag=f"kt{h}")
                    for h in range(HPC)
                ]
                for h in range(HPC):
                    if b < 2:  # qk pool has 2 buffers; row 64 persists after
                        nc.vector.memset(qt[h][DH : DH + 1, :].bitcast(U16), 0x3F80)
                    nc.sync.dma_start(kt[h][DH : DH + 1, :], biask_d[b : b + 1, :])
                return qt, kt

            def load_xt(b):
                # one coalesced DMA: chunk i of x^T lands at cols [i*T,(i+1)*T)
                xt_t = xtb_pool.tile([128, 8 * T], BF16, name="xtb", tag="xtb")
                nc.sync.dma_start(
                    xt_t[:].rearrange("p (i t) -> p i t", i=8),
                    xt_d[b].rearrange("i p t -> p i t"),
                )
                return xt_t

            # weights/x first (the first qk chain needs wq half 0 + x chunk
            # 0); consts later — nothing reads them for ~15us
            first_xt = []
            nc.sync.dma_start(wq_sb[:, 0 : 4 * WCH], wqkv_d[:, 0 : 4 * WCH])
            for i in range(4):
                xt_t = xt_pool.tile([128, T], BF16, name="xt", tag="xt")
                nc.sync.dma_start(xt_t[:], xt_d[0, i, :, :])
                first_xt.append(xt_t)
            nc.sync.dma_start(wq_sb[:, 4 * WCH :], wqkv_d[:, 4 * WCH :])
            for i in range(4, 8):
                xt_t = xt_pool.tile([128, T], BF16, name="xt", tag="xt")
                nc.sync.dma_start(xt_t[:], xt_d[0, i, :, :])
                first_xt.append(xt_t)
            nc.sync.dma_start(bqb[:], bqb_d[:])
            nc.sync.dma_start(bkb[:], bkb_d[:])
            nc.sync.dma_start(bvb[:], bvb_d[:])
            nc.sync.dma_start(tri[:], tri_d[:])
            nc.sync.dma_start(wp_sb[:], wproj_d[:])

            # warm the Act exp table while startup DMAs are in flight
            warm = cpool.tile([1, 1], F32, name="warm", tag="warm")
            nc.scalar.activation(warm[:], warm[:], EXP)

            cur_xt = first_xt
            cur_qt, cur_kt = load_qk(0)

            def proj(fin, pb):
                """Partial output projection for batch pb's 1024 rows.
                Drains alternate Act/DVE; one coalesced store per tb."""
                for tb in range(8):
                    y_sb = y_pool.tile([128, T], BF16, name="ysb", tag="ysb")
                    for ch in range(2):
                        csl = slice(ch * 512, (ch + 1) * 512)
                        y_ps = y_ps_pool.tile([128, 512], F32, name="yps", tag="yps")
                        nc.tensor.matmul(
                            y_ps[:],
                            fin[:, tb * 128 : (tb + 1) * 128],
                            wp_sb[:, csl],
                            start=True,
                            stop=True,
                        )
                        if ch == 0:
                            nc.vector.tensor_copy(y_sb[:, csl], y_ps[:])
                        else:
                            nc.scalar.activation(y_sb[:, csl], y_ps[:], COPY)
                    nc.sync.dma_start(
                        out_d[pb * T + tb * 128 : pb * T + (tb + 1) * 128, :],
                        y_sb[:],
                    )

            def norm_ch(h, o_ps, att_fin, ch):
                """Normalize o_ps[ch] rows by the folded row-sum (row DH).
                NB: DVE ops straight off a PSUM row at partition offset 64
                read the wrong row — bounce through Act."""
                csl = slice(ch * 512, (ch + 1) * 512)
                ssum = nrm_pool.tile([1, 512], F32, name="ssum", tag="ssum")
                recip = nrm_pool.tile([1, 512], F32, name="recip", tag="recip")
                bcast = nrm_pool.tile([DH, 512], F32, name="bcast", tag="bcast")
                nc.scalar.activation(ssum[:], o_ps[ch][DH : DH + 1, :], COPY)
                nc.vector.reciprocal_approx_fast(recip[:], ssum[:])
                nc.gpsimd.partition_broadcast(bcast[:], recip[:])
                nc.vector.tensor_tensor(
                    out=att_fin[h * DH : (h + 1) * DH, csl],
                    in0=o_ps[ch][0:DH, :],
                    in1=bcast[:],
                    op=MULT,
                )

            prev_fin, prev_b = None, None

            for b in range(B):
                xt_sb = cur_xt
                qt, kt = cur_qt, cur_kt
                if b == 0:
                    xs = lambda i, lo, hi: xt_sb[i][:, lo:hi]
                else:
                    xs = lambda i, lo, hi: xt_sb[:, i * T + lo : i * T + hi]

                # ---- q,k projections (transposed: [f, t]) ----
                for fb in range(2):
                    bias = bqb if fb == 0 else bkb
                    dst = qt if fb == 0 else kt
                    for ch in range(2):
                        csl = slice(ch * 512, (ch + 1) * 512)
                        mm_ps = mm_ps_pool.tile(
                            [128, 512], F32, name="mm512", tag="mm512"
                        )
                        for i in range(8):
                            nc.tensor.matmul(
                                mm_ps[:],
                                wq_sb[:, i * WCH + fb * 128 : i * WCH + (fb + 1) * 128],
                                xs(i, ch * 512, (ch + 1) * 512),
                                start=(i == 0),
                                stop=(i == 7),
                            )
                        for h in range(HPC):
                            nc.vector.tensor_tensor(
                                out=dst[h][0:DH, csl],
                                in0=mm_ps[h * DH : (h + 1) * DH, :],
                                in1=bias[h * DH : (h + 1) * DH, csl],
                                op=ADD,
                            )

                # ---- prefetch next batch's x / qk tiles first: their DMA
                # triggers must not sit behind the proj stores' semaphore
                # waits on the sync queue ----
                if b + 1 < B:
                    cur_xt = load_xt(b + 1)
                    cur_qt, cur_kt = load_qk(b + 1)

                # ---- proj of the previous batch: ready PE work that hides
                # the previous normalize + drain latency ----
                if prev_fin is not None:
                    proj(prev_fin, prev_b)

                # ---- v directly in [t, f] layout ----
                v_sb = []
                for tb in range(8):
                    mm_ps = mm_ps_pool.tile([128, 512], F32, name="mm512", tag="mm512")
                    for i in range(8):
                        nc.tensor.matmul(
                            mm_ps[:, 0:130],
                            xs(i, tb * 128, (tb + 1) * 128),
                            wq_sb[:, i * WCH + 256 : (i + 1) * WCH],
                            start=(i == 0),
                            stop=(i == 7),
                        )
                    vs = vs_pool.tile([128, 130], BF16, name="vsb", tag="vsb")
                    nc.vector.tensor_tensor(
                        out=vs[:], in0=mm_ps[:, 0:130], in1=bvb[:], op=ADD
                    )
                    v_sb.append(vs)

                # ---- attention per head ----
                att_fin = fin_pool.tile([128, T], BF16, name="fin", tag="fin")
                for h in range(HPC):
                    o_ps = [
                        o_ps_pool.tile([DH + 1, 512], F32, name="o_ps", tag="o_ps")
                        for _ in range(2)
                    ]
                    atts = []

                    def av(kb):
                        # AV lags scores by 2 kb-steps: exp+tri latency is
                        # hidden without draining the PE queue
                        k0 = kb * 128
                        att = atts[kb]
                        for ch in range(2):
                            lo = max(k0, ch * 512)
                            hi = (ch + 1) * 512
                            if lo >= hi:
                                continue
                            nc.tensor.matmul(
                                o_ps[ch][:, lo - ch * 512 : hi - ch * 512],
                                v_sb[kb][:, h * 65 : (h + 1) * 65],
                                att[:, lo - k0 : hi - k0],
                                start=(kb == 0),
                                stop=(kb == (3 if ch == 0 else 7)),
                            )

                    for kb in range(8):
                        k0 = kb * 128
                        width = T - k0
                        att = att_pool.tile([128, T], BF16, name="att", tag="att")
                        for off, w in _chunks(width):
                            s_ps = s_ps_pool.tile(
                                [128, 512], F32, name="s_ps", tag="s_ps"
                            )
                            nc.tensor.matmul(
                                s_ps[:, 0:w],
                                kt[h][:, k0 : k0 + 128],
                                qt[h][:, k0 + off : k0 + off + w],
                                start=True,
                                stop=True,
                            )
                            nc.scalar.activation(
                                att[:, off : off + w], s_ps[:, 0:w], EXP
                            )
                        # causal mask on the diagonal block
                        nc.vector.tensor_tensor(
                            out=att[:, 0:128],
                            in0=att[:, 0:128],
                            in1=tri[:],
                            op=MULT,
                        )
                        atts.append(att)
                        if kb >= 3:
                            av(kb - 3)
                    av(5)
                    av(6)
                    av(7)
                    norm_ch(h, o_ps, att_fin, 0)
                    norm_ch(h, o_ps, att_fin, 1)

                prev_fin, prev_b = att_fin, b

            proj(prev_fin, prev_b)

    nc.compile()
    return nc


def _get_nc():
    if "nc" not in _CACHE:
        _CACHE["nc"] = _build()
    return _CACHE["nc"]


def kernel(x, Wqkv, bqkv, Wproj, bproj, mask):
    import ml_dtypes
    from concourse.bass_utils import run_bass_kernel_spmd

    BF = ml_dtypes.bfloat16
    x = np.asarray(x, dtype=np.float32)
    Wqkv = np.asarray(Wqkv, dtype=np.float32)
    bqkv = np.asarray(bqkv, dtype=np.float32)
    Wproj = np.asarray(Wproj, dtype=np.float32)
    bproj = np.asarray(bproj, dtype=np.float32)
    mask = np.asarray(mask)

    nc = _get_nc()

    # [B, 8, 128, T]: d-chunk-major view of x^T
    xt = np.ascontiguousarray(x.transpose(0, 2, 1)).astype(BF).reshape(B, 8, 128, T)
    biask = np.where(mask == 0, np.float32(-30000.0), np.float32(0.0)).astype(BF)
    tri = np.triu(np.ones((128, 128), np.float32)).astype(BF)

    in_maps = []
    for c in range(NC):
        cols = slice(c * HPC * DH, (c + 1) * HPC * DH)  # this core's head features
        wq = Wqkv[:, 0:D][:, cols] * 0.125  # scores scale folded into Wq/bq
        wk = Wqkv[:, D : 2 * D][:, cols]
        wv = Wqkv[:, 2 * D : 3 * D][:, cols]
        # per-chunk layout [wq(128) | wk(128) | wv_h0(64) 0 wv_h1(64) 0]
        wv_aug = np.zeros((D, 130), np.float32)
        wv_aug[:, 0:DH] = wv[:, 0:DH]
        wv_aug[:, DH + 1 : 2 * DH + 1] = wv[:, DH : 2 * DH]
        # pre-chunked [128, 8*WCH]: chunk i rows i*128:(i+1)*128 side by side
        w_local = (
            np.concatenate([wq, wk, wv_aug], axis=1)
            .astype(BF)
            .reshape(8, 128, WCH)
            .transpose(1, 0, 2)
            .reshape(128, 8 * WCH)
        )
        w_local = np.ascontiguousarray(w_local)
        bq = bqkv[0:D][cols] * 0.125
        bk = bqkv[D : 2 * D][cols]
        bv = bqkv[2 * D : 3 * D][cols]
        bqb = np.repeat(bq[:, None], T, axis=1).astype(BF)  # [128, T]
        bkb = np.repeat(bk[:, None], T, axis=1).astype(BF)
        bvrow = np.zeros(130, np.float32)
        bvrow[0:DH] = bv[0:DH]
        bvrow[DH] = 1.0
        bvrow[DH + 1 : 2 * DH + 1] = bv[DH : 2 * DH]
        bvrow[2 * DH + 1] = 1.0
        bvb = np.broadcast_to(bvrow, (128, 130)).copy()
        in_maps.append(
            {
                "xt": xt,
                "wqkv": w_local,
                "bqb": bqb,
                "bkb": bkb,
                "bvb": bvb,
                "biask": biask,
                "wproj": Wproj[c * 128 : (c + 1) * 128, :].astype(BF),
                "tri": tri,
            }
        )

    res = run_bass_kernel_spmd(nc, in_maps, core_ids=list(range(NC)))
    # out_c = att_local^T @ Wproj_local; full out = sum_c out_c + bproj
    y = res.results[0]["out"].astype(np.float32)
    for c in range(1, NC):
        y += res.results[c]["out"].astype(np.float32)
    y = (y + bproj).astype(np.float32)
    return y.reshape(B, T, D)
